# revision 1
# baseline (speedup 1.0000x reference)
"""DeepPoly ReLU transformer (back-substitution concretization) on 8 trn2 cores.

Math (exact rewrite of the reference):
    lb, ub = bounds;  plb, pub = last_bounds
    c = (plb+pub)/2, r = (pub-plb)/2  (r >= 0 for sorted bounds)
    s = W @ c,  q = |W| @ r
    A = s - q  ( = max(W,0)@plb + min(W,0)@pub )
    B = s + q  ( = max(W,0)@pub + min(W,0)@plb )
    ind2 = lb>=0; ind3 = (ub>0)&(lb<0); ind4 = (ub>-lb)&ind3
    beta = 1 if ind2|ind4 else 0
    lmbda = 1 if ind2 else (ub/(ub-lb) if ind3 else 0)
    mu    = -lb*ub/(ub-lb) if ind3 else 0
    low = beta*(A + bias);  up = lmbda*(B + bias) + mu
    out_lb = max(beta*lb, low)
    out_ub = min(where(ind2|ind3, ub, 0), up)

Sharding: rows of W (output neurons) split across 8 cores, 1024 rows each.
Per core, W rows stream HBM->SBUF via HWDGE as fp32 [128, 4096] chunks;
ScalarE casts them to fp16 (activation Copy — ScalarE work overlaps DMA on
this system, DVE work on DMA-dependent data does not). DVE computes
P = W16*c_b and Q = W16*r_b (tensor_tensor, fp16 2x mode) and the row-sum
s = sum(P) via tensor_scalar+accum_out (4x mode); ScalarE folds
q = sum(|Q|) via activation(Abs, accum_out). The [2,1024] epilogue
(masks/lmbda/mu/final max-min) is exact fp32 on DVE.

Measured 167-178us/shot (+-5us inter-run drift) vs a ~168us analytic floor
on this axon backend: ~100us fp32 W streaming (96% of the per-core HBM cap)
plus ~68us of DVE product time that this system does not overlap with DMA
(ScalarE work and independent DVE work hide; DVE ops consuming loaded data
serialize — established by ablation since NTFF profiling is unavailable).
"""

import sys

sys.path.insert(0, "/opt/trn_rl_repo")

import numpy as np

N_CORES = 8
N = 8192
M = 8192


def _build(
    rows_per_core: int,
    m: int,
    concretize: bool,
    general_r: bool,
    rep: int = 1,
    split_sred: bool = False,
    col_chunks: int = 1,
    ablate: str = "",
    cast_mode: str = "dma",
    rep_static: bool = False,
    wbufs: int = 3,
    pbufs: int = 3,
):
    import contextlib

    import concourse.tile as tile
    from concourse import bacc, mybir

    T = rows_per_core // 128
    assert rows_per_core % 128 == 0

    nc = bacc.Bacc("TRN2", target_bir_lowering=False, debug=False)
    f32 = mybir.dt.float32
    f16 = mybir.dt.float16
    Alu = mybir.AluOpType
    Act = mybir.ActivationFunctionType

    lbt = nc.dram_tensor("lbt", [128, T], f32, kind="ExternalInput").ap()
    ubt = nc.dram_tensor("ubt", [128, T], f32, kind="ExternalInput").ap()
    olb = nc.dram_tensor("olb", [128, T], f32, kind="ExternalOutput").ap()
    oub = nc.dram_tensor("oub", [128, T], f32, kind="ExternalOutput").ap()
    if concretize:
        w = nc.dram_tensor("w", [rows_per_core, m], f32, kind="ExternalInput").ap()
        biast = nc.dram_tensor("biast", [128, T], f32, kind="ExternalInput").ap()
        cb = nc.dram_tensor("cb", [1, m], f16, kind="ExternalInput").ap()
        rb = nc.dram_tensor("rb", [1, m], f16, kind="ExternalInput").ap()
        if general_r:
            rbn = nc.dram_tensor("rbn", [1, m], f16, kind="ExternalInput").ap()

    with tile.TileContext(nc) as tc:
        with (
            tc.tile_pool(name="wp", bufs=wbufs) as wp,
            tc.tile_pool(name="wq", bufs=wbufs) as wq,
            tc.tile_pool(name="pp", bufs=pbufs) as pp,
            tc.tile_pool(name="dp", bufs=pbufs) as dp,
            tc.tile_pool(name="bc", bufs=1) as bc,
            tc.tile_pool(name="sm", bufs=1) as sm,
            tc.tile_pool(name="ep", bufs=24) as ep,
            tc.For_i(0, rep, 1) if (rep > 1 and not rep_static) else contextlib.nullcontext(),
        ):
          for _rep_i in range(rep if rep_static else 1):
            lb_s = sm.tile([128, T], f32, tag="lb")
            ub_s = sm.tile([128, T], f32, tag="ub")
            nc.sync.dma_start(lb_s[:], lbt[:])
            nc.sync.dma_start(ub_s[:], ubt[:])

            if concretize:
                # broadcast c/r row vectors to all 128 partitions (log2 doubling)
                cb_b = bc.tile([128, m], f16, tag="cbb")
                rb_b = bc.tile([128, m], f16, tag="rbb")
                nc.sync.dma_start(cb_b[0:1, :], cb[:])
                nc.sync.dma_start(rb_b[0:1, :], rb[:])
                nc.gpsimd.partition_broadcast(cb_b[:], cb_b[0:1, :])
                nc.gpsimd.partition_broadcast(rb_b[:], rb_b[0:1, :])
                if general_r:
                    rbn_b = bc.tile([128, m], f16, tag="rbnb")
                    nc.sync.dma_start(rbn_b[0:1, :], rbn[:])
                    nc.gpsimd.partition_broadcast(rbn_b[:], rbn_b[0:1, :])

                bias_s = sm.tile([128, T], f32, tag="bias")
                nc.sync.dma_start(bias_s[:], biast[:])

                # accumulators: chunk-major columns [j*T + t]
                CC = col_chunks
                mc = m // CC
                s_all = sm.tile([128, T * CC], f32, tag="sall")
                q_all = sm.tile([128, T * CC], f32, tag="qall")
                if general_r:
                    qn_all = sm.tile([128, T * CC], f32, tag="qnall")

                if ablate:
                    nc.vector.memset(s_all[:], 0.0)
                    nc.vector.memset(q_all[:], 0.0)

                for t in range(T):
                    for j in range(CC):
                        col = j * T + t
                        cs = slice(j * mc, (j + 1) * mc)
                        if ablate == "dma32":
                            w32 = wp.tile([128, mc], f32, tag="w32")
                            nc.sync.dma_start(
                                w32[:], w[t * 128 : (t + 1) * 128, cs]
                            )
                            continue
                        if ablate == "dvefree":
                            # HWDGE loads + DVE TTs on tiles UNRELATED to the
                            # loads: probes whether DVE overlaps DMA at all.
                            w32 = wp.tile([128, mc], f32, tag="w32")
                            nc.sync.dma_start(
                                w32[:], w[t * 128 : (t + 1) * 128, cs]
                            )
                            pf = pp.tile([128, mc], f16, tag="prod")
                            nc.vector.tensor_tensor(
                                pf[:], cb_b[:, cs], rb_b[:, cs], op=Alu.mult
                            )
                            qf = pp.tile([128, mc], f16, tag="prod")
                            nc.vector.tensor_tensor(
                                qf[:], cb_b[:, cs], rb_b[:, cs], op=Alu.mult
                            )
                            continue
                        w16 = wq.tile([128, mc], f16, tag="w16")
                        if cast_mode == "dma":
                            nc.gpsimd.dma_start(
                                w16[:], w[t * 128 : (t + 1) * 128, cs]
                            )
                        else:
                            w32 = wp.tile([128, mc], f32, tag="w32")
                            nc.sync.dma_start(
                                w32[:], w[t * 128 : (t + 1) * 128, cs]
                            )
                            if cast_mode == "dve":
                                nc.vector.tensor_copy(w16[:], w32[:])
                            elif cast_mode == "gpsimd":
                                nc.gpsimd.tensor_copy(w16[:], w32[:])
                            elif cast_mode == "act":
                                nc.scalar.activation(w16[:], w32[:], Act.Copy)
                            elif cast_mode == "dve_act":
                                h = mc // 2
                                nc.vector.tensor_copy(w16[:, 0:h], w32[:, 0:h])
                                nc.scalar.activation(
                                    w16[:, h:mc], w32[:, h:mc], Act.Copy
                                )
                            else:
                                raise ValueError(cast_mode)
                        if ablate == "dma":
                            continue
                        if ablate == "nodve":
                            d2 = dp.tile([128, mc], f16, tag="dump")
                            nc.scalar.activation(
                                d2[:], w16[:], Act.Abs,
                                accum_out=q_all[:, col : col + 1],
                            )
                            continue

                        p = pp.tile([128, mc], f16, tag="prod")
                        nc.vector.tensor_tensor(p[:], w16[:], cb_b[:, cs], op=Alu.mult)
                        d1 = dp.tile([128, mc], f16, tag="dump")
                        if split_sred and (t * CC + j) % 2 == 0 and t < T - 1:
                            # balance: put every other chunk's s-reduce on ACT
                            nc.scalar.activation(
                                d1[:], p[:], Act.Copy,
                                accum_out=s_all[:, col : col + 1],
                            )
                        else:
                            nc.vector.tensor_scalar(
                                d1[:], p[:], 0.0, None, Alu.add, Alu.add,
                                accum_out=s_all[:, col : col + 1],
                            )

                        qt = pp.tile([128, mc], f16, tag="prod")
                        qt_eng = nc.gpsimd if ablate == "qpool" else nc.vector
                        qt_eng.tensor_tensor(qt[:], w16[:], rb_b[:, cs], op=Alu.mult)
                        if ablate != "noact":
                            d2 = dp.tile([128, mc], f16, tag="dump")
                            nc.scalar.activation(
                                d2[:], qt[:], Act.Abs,
                                accum_out=q_all[:, col : col + 1],
                            )

                        if general_r:
                            qn = pp.tile([128, mc], f16, tag="prod")
                            nc.vector.tensor_tensor(
                                qn[:], w16[:], rbn_b[:, cs], op=Alu.mult
                            )
                            d3 = dp.tile([128, mc], f16, tag="dump")
                            nc.scalar.activation(
                                d3[:], qn[:], Act.Abs,
                                accum_out=qn_all[:, col : col + 1],
                            )

            # ---------------- epilogue (all fp32, [128, T]) ----------------
            def tt(a, b, op):
                o = ep.tile([128, T], f32)
                nc.vector.tensor_tensor(o[:], a[:], b[:], op=op)
                return o

            def ts(a, s1, op0, s2=None, op1=None):
                o = ep.tile([128, T], f32)
                if op1 is None:
                    nc.vector.tensor_scalar(o[:], a[:], s1, None, op0)
                else:
                    nc.vector.tensor_scalar(o[:], a[:], s1, s2, op0, op1)
                return o

            ind2 = ts(lb_s, 0.0, Alu.is_ge)
            ubpos = ts(ub_s, 0.0, Alu.is_gt)
            lbneg = ts(lb_s, 0.0, Alu.is_lt)
            ind3 = tt(ubpos, lbneg, Alu.mult)
            sumlu = tt(ub_s, lb_s, Alu.add)
            ind4p = ts(sumlu, 0.0, Alu.is_gt)
            ind4 = tt(ind4p, ind3, Alu.mult)
            beta = tt(ind2, ind4, Alu.max)

            lb_pre = tt(beta, lb_s, Alu.mult)
            i23 = tt(ind2, ind3, Alu.max)
            ub_pre = tt(ub_s, i23, Alu.mult)

            if concretize:
                diff = tt(ub_s, lb_s, Alu.subtract)
                dmask = tt(diff, ind3, Alu.mult)
                onemind3 = ts(ind3, -1.0, Alu.mult, 1.0, Alu.add)  # 1 - ind3
                diff_safe = tt(dmask, onemind3, Alu.add)
                rec = ep.tile([128, T], f32)
                nc.vector.reciprocal(rec[:], diff_safe[:])
                ubrec = tt(ub_s, rec, Alu.mult)
                lmb3 = tt(ubrec, ind3, Alu.mult)
                lmbda = tt(ind2, lmb3, Alu.add)
                negmu = tt(lmb3, lb_s, Alu.mult)  # = -mu

                def fold(acc):
                    if CC == 1:
                        return acc
                    o = tt(acc[:, 0:T], acc[:, T : 2 * T], Alu.add)
                    for j in range(2, CC):
                        o = tt(o, acc[:, j * T : (j + 1) * T], Alu.add)
                    return o

                s_eff = fold(s_all)
                q_eff = fold(q_all)
                if general_r:
                    q_eff = tt(q_eff, fold(qn_all), Alu.subtract)
                a_lo = tt(s_eff, q_eff, Alu.subtract)
                b_up = tt(s_eff, q_eff, Alu.add)
                a_b = tt(a_lo, bias_s, Alu.add)
                low = tt(a_b, beta, Alu.mult)
                b_b = tt(b_up, bias_s, Alu.add)
                b_l = tt(b_b, lmbda, Alu.mult)
                up = tt(b_l, negmu, Alu.subtract)

                out_lb = tt(lb_pre, low, Alu.max)
                out_ub = tt(ub_pre, up, Alu.min)
            else:
                out_lb = lb_pre
                out_ub = ub_pre

            nc.sync.dma_start(olb[:], out_lb[:])
            nc.sync.dma_start(oub[:], out_ub[:])

    nc.compile()
    return nc


_cache: dict = {}


def _get_nc(
    rows_per_core: int,
    m: int,
    concretize: bool,
    general_r: bool,
    rep: int = 1,
    split_sred: bool = False,
    col_chunks: int = 1,
    ablate: str = "",
    cast_mode: str = "dma",
    rep_static: bool = False,
    wbufs: int = 3,
    pbufs: int = 3,
):
    key = (rows_per_core, m, concretize, general_r, rep, split_sred, col_chunks,
           ablate, cast_mode, rep_static, wbufs, pbufs)
    if key not in _cache:
        _cache[key] = _build(
            rows_per_core, m, concretize, general_r, rep, split_sred, col_chunks,
            ablate, cast_mode, rep_static, wbufs, pbufs,
        )
    return _cache[key]


def _make_in_maps(bounds, W, bias, last_bounds, concretize, general_r, n_cores):
    rows = W.shape[0] // n_cores if W is not None else bounds.shape[1] // n_cores
    T = rows // 128
    lb, ub = np.asarray(bounds[0], np.float32), np.asarray(bounds[1], np.float32)
    in_maps = []
    if concretize:
        plb = np.asarray(last_bounds[0], np.float64)
        pub = np.asarray(last_bounds[1], np.float64)
        c = ((plb + pub) * 0.5).astype(np.float32)
        r = ((pub - plb) * 0.5).astype(np.float32)
        if general_r:
            rpos = np.maximum(r, 0.0)
            rneg = np.minimum(r, 0.0)
            cb16 = c.astype(np.float16)[None, :]
            rb16 = rpos.astype(np.float16)[None, :]
            rbn16 = (-rneg).astype(np.float16)[None, :]
        else:
            cb16 = c.astype(np.float16)[None, :]
            rb16 = r.astype(np.float16)[None, :]
            rbn16 = None
    for cix in range(n_cores):
        sl = slice(cix * rows, (cix + 1) * rows)
        im = {
            "lbt": np.ascontiguousarray(lb[sl].reshape(T, 128).T),
            "ubt": np.ascontiguousarray(ub[sl].reshape(T, 128).T),
        }
        if concretize:
            im["w"] = np.ascontiguousarray(W[sl])
            im["biast"] = np.ascontiguousarray(
                np.asarray(bias, np.float32)[sl].reshape(T, 128).T
            )
            im["cb"] = cb16
            im["rb"] = rb16
            if general_r:
                im["rbn"] = rbn16
        in_maps.append(im)
    return in_maps


def _assemble(results, n_cores):
    outs = []
    for cix in range(n_cores):
        o_lb = results[cix]["olb"].T.reshape(-1)  # [T,128] -> rows t*128+p
        o_ub = results[cix]["oub"].T.reshape(-1)
        outs.append(np.stack([o_lb, o_ub]))
    return np.concatenate(outs, axis=1).astype(np.float32)


# best measured configuration (see bench.py results)
BEST = dict(split_sred=False, col_chunks=2, cast_mode="act")


def kernel(bounds, W, bias, last_bounds, back_sub_steps):
    from concourse.bass_utils import run_bass_kernel_spmd

    bounds = np.asarray(bounds)
    W = np.asarray(W)
    bias = np.asarray(bias)
    last_bounds = np.asarray(last_bounds)
    concretize = int(np.asarray(back_sub_steps)) > 0

    general_r = False
    if concretize:
        r = (last_bounds[1].astype(np.float64) - last_bounds[0].astype(np.float64))
        general_r = bool((r < 0).any())

    rows = W.shape[0] // N_CORES
    nc = _get_nc(rows, W.shape[1], concretize, general_r, **BEST)
    in_maps = _make_in_maps(
        bounds, W if concretize else None, bias, last_bounds,
        concretize, general_r, N_CORES,
    )
    res = run_bass_kernel_spmd(nc, in_maps, list(range(N_CORES)))
    return _assemble(res.results, N_CORES)


if __name__ == "__main__":
    rng = np.random.default_rng(0)
    n, m = 1024, 2048  # small smoke (1 core slice = 128 rows)
    bounds = np.sort(rng.standard_normal((2, n)).astype(np.float32), axis=0)
    W = (rng.standard_normal((n, m)) / np.sqrt(m)).astype(np.float32)
    bias = rng.standard_normal(n).astype(np.float32)
    last_bounds = np.sort(rng.standard_normal((2, m)).astype(np.float32), axis=0)
    out = kernel(bounds, W, bias, last_bounds, 1)
    print(out.shape, out.dtype)



# revision 7
# speedup vs baseline: 2.1069x; 2.1069x over previous
"""DeepPoly ReLU transformer (back-substitution concretization) on 8 trn2 cores.

Math (exact rewrite of the reference):
    lb, ub = bounds;  plb, pub = last_bounds
    c = (plb+pub)/2, r = (pub-plb)/2
    s = W @ c,  q = |W| @ r       (identity holds for any sign of r)
    A = s - q  ( = max(W,0)@plb + min(W,0)@pub )
    B = s + q  ( = max(W,0)@pub + min(W,0)@plb )
    ind2 = lb>=0; ind3 = (ub>0)&(lb<0); ind4 = (ub>-lb)&ind3
    beta = 1 if ind2|ind4 else 0
    lmbda = 1 if ind2 else (ub/(ub-lb) if ind3 else 0)
    mu    = -lb*ub/(ub-lb) if ind3 else 0
    low = beta*(A + bias);  up = lmbda*(B + bias) + mu
    out_lb = max(beta*lb, low)
    out_ub = min(where(ind2|ind3, ub, 0), up)

Sharding: rows of W (output neurons) split across 8 cores, 1024 rows each.
Per core the host ships W transposed as fp16 [8192, 1024] (halves HBM
traffic vs fp32 and puts the contraction dim on partitions), so both
matvecs run on the TensorEngine: per [128, 1024] chunk, DVE computes
|Wt| (tensor_scalar abs_max, 4x mode), then PE accumulates
s = Wt.T @ c_chunk and q = |Wt|.T @ r_chunk into PSUM ([1,512] banks,
start/stop over the 64 chunks).  The [1, 1024] PSUM rows are scattered
to [128, 8] by a small SBUF->SBUF DMA and the exact fp32 mask epilogue
(same as the previous DVE version) finishes on [128, 8] tiles.
"""

import sys

sys.path.insert(0, "/opt/trn_rl_repo")

import numpy as np

N_CORES = 8
N = 8192
M = 8192


def _build(
    rows_per_core: int,
    m: int,
    concretize: bool,
    rep: int = 1,
    wbufs: int = 4,
):
    import contextlib

    import concourse.tile as tile
    from concourse import bacc, mybir

    T = rows_per_core // 128
    assert rows_per_core % 128 == 0
    JC = m // 128  # number of 128-row contraction chunks

    nc = bacc.Bacc("TRN2", target_bir_lowering=False, debug=False)
    f32 = mybir.dt.float32
    f16 = mybir.dt.float16
    Alu = mybir.AluOpType

    lbt = nc.dram_tensor("lbt", [128, T], f32, kind="ExternalInput").ap()
    ubt = nc.dram_tensor("ubt", [128, T], f32, kind="ExternalInput").ap()
    olb = nc.dram_tensor("olb", [128, T], f32, kind="ExternalOutput").ap()
    oub = nc.dram_tensor("oub", [128, T], f32, kind="ExternalOutput").ap()
    if concretize:
        wt = nc.dram_tensor("wt", [m, rows_per_core], f16, kind="ExternalInput").ap()
        biast = nc.dram_tensor("biast", [128, T], f32, kind="ExternalInput").ap()
        ct = nc.dram_tensor("ct", [128, JC], f16, kind="ExternalInput").ap()
        rt = nc.dram_tensor("rt", [128, JC], f16, kind="ExternalInput").ap()
        # DRAM scratch for the [1, rows] -> [128, T] partition scatter
        sdram = nc.dram_tensor("sdram", [T, 128], f32, kind="Internal").ap()
        qdram = nc.dram_tensor("qdram", [T, 128], f32, kind="Internal").ap()

    # psum banks per accumulator: free dim <= 512 fp32 (one 2KB bank) each
    bank_slices = [
        slice(b0, min(b0 + 512, rows_per_core))
        for b0 in range(0, rows_per_core, 512)
    ]
    NB = len(bank_slices)

    with tile.TileContext(nc) as tc:
        with (
            tc.tile_pool(name="wp", bufs=wbufs) as wp,
            tc.tile_pool(name="aq", bufs=wbufs) as aq,
            tc.tile_pool(name="ps", bufs=1, space="PSUM") as ps,
            tc.tile_pool(name="bc", bufs=1) as bc,
            tc.tile_pool(name="sm", bufs=1) as sm,
            tc.tile_pool(name="ep", bufs=24) as ep,
            tc.For_i(0, rep, 1) if rep > 1 else contextlib.nullcontext(),
        ):
            lb_s = sm.tile([128, T], f32, tag="lb")
            ub_s = sm.tile([128, T], f32, tag="ub")
            nc.sync.dma_start(lb_s[:], lbt[:])
            nc.sync.dma_start(ub_s[:], ubt[:])

            if concretize:
                ct_s = bc.tile([128, JC], f16, tag="ct")
                rt_s = bc.tile([128, JC], f16, tag="rt")
                nc.sync.dma_start(ct_s[:], ct[:])
                nc.sync.dma_start(rt_s[:], rt[:])
                bias_s = sm.tile([128, T], f32, tag="bias")
                nc.sync.dma_start(bias_s[:], biast[:])

                s_ps = [
                    ps.tile([1, cs.stop - cs.start], f32, name=f"sps{b}", tag=f"sps{b}")
                    for b, cs in enumerate(bank_slices)
                ]
                q_ps = [
                    ps.tile([1, cs.stop - cs.start], f32, name=f"qps{b}", tag=f"qps{b}")
                    for b, cs in enumerate(bank_slices)
                ]

                for jc in range(JC):
                    w = wp.tile([128, rows_per_core], f16, tag="w")
                    nc.sync.dma_start(w[:], wt[jc * 128 : (jc + 1) * 128, :])
                    a = aq.tile([128, rows_per_core], f16, tag="a")
                    # fp16 abs = clear the sign bit (int16 view): DVE 4x mode
                    nc.vector.tensor_scalar(
                        a[:].bitcast(mybir.dt.int16),
                        w[:].bitcast(mybir.dt.int16),
                        0x7FFF, None, Alu.bitwise_and,
                    )
                    st, sp = jc == 0, jc == JC - 1
                    for b, cs in enumerate(bank_slices):
                        nc.tensor.matmul(
                            s_ps[b][:], ct_s[:, jc : jc + 1], w[:, cs],
                            start=st, stop=sp,
                        )
                        nc.tensor.matmul(
                            q_ps[b][:], rt_s[:, jc : jc + 1], a[:, cs],
                            start=st, stop=sp,
                        )

                # PSUM [1, 512] rows -> SBUF [1, rows] -> scatter to [128, T]
                s_row = sm.tile([1, rows_per_core], f32, tag="srow")
                q_row = sm.tile([1, rows_per_core], f32, tag="qrow")
                for b, cs in enumerate(bank_slices):
                    nc.vector.tensor_copy(s_row[0:1, cs], s_ps[b][:])
                    nc.vector.tensor_copy(q_row[0:1, cs], q_ps[b][:])
                s_pt = sm.tile([128, T], f32, tag="spt")
                q_pt = sm.tile([128, T], f32, tag="qpt")
                nc.sync.dma_start(sdram[:], s_row[0:1, :])
                nc.sync.dma_start(qdram[:], q_row[0:1, :])
                nc.sync.dma_start(s_pt[:], sdram.rearrange("t p -> p t"))
                nc.sync.dma_start(q_pt[:], qdram.rearrange("t p -> p t"))

            # ---------------- epilogue (all fp32, [128, T]) ----------------
            def tt(a, b, op):
                o = ep.tile([128, T], f32)
                nc.vector.tensor_tensor(o[:], a[:], b[:], op=op)
                return o

            def ts(a, s1, op0, s2=None, op1=None):
                o = ep.tile([128, T], f32)
                if op1 is None:
                    nc.vector.tensor_scalar(o[:], a[:], s1, None, op0)
                else:
                    nc.vector.tensor_scalar(o[:], a[:], s1, s2, op0, op1)
                return o

            ind2 = ts(lb_s, 0.0, Alu.is_ge)
            ubpos = ts(ub_s, 0.0, Alu.is_gt)
            lbneg = ts(lb_s, 0.0, Alu.is_lt)
            ind3 = tt(ubpos, lbneg, Alu.mult)
            sumlu = tt(ub_s, lb_s, Alu.add)
            ind4p = ts(sumlu, 0.0, Alu.is_gt)
            ind4 = tt(ind4p, ind3, Alu.mult)
            beta = tt(ind2, ind4, Alu.max)

            lb_pre = tt(beta, lb_s, Alu.mult)
            i23 = tt(ind2, ind3, Alu.max)
            ub_pre = tt(ub_s, i23, Alu.mult)

            if concretize:
                diff = tt(ub_s, lb_s, Alu.subtract)
                dmask = tt(diff, ind3, Alu.mult)
                onemind3 = ts(ind3, -1.0, Alu.mult, 1.0, Alu.add)  # 1 - ind3
                diff_safe = tt(dmask, onemind3, Alu.add)
                rec = ep.tile([128, T], f32)
                nc.vector.reciprocal(rec[:], diff_safe[:])
                ubrec = tt(ub_s, rec, Alu.mult)
                lmb3 = tt(ubrec, ind3, Alu.mult)
                lmbda = tt(ind2, lmb3, Alu.add)
                negmu = tt(lmb3, lb_s, Alu.mult)  # = -mu

                a_lo = tt(s_pt, q_pt, Alu.subtract)
                b_up = tt(s_pt, q_pt, Alu.add)
                a_b = tt(a_lo, bias_s, Alu.add)
                low = tt(a_b, beta, Alu.mult)
                b_b = tt(b_up, bias_s, Alu.add)
                b_l = tt(b_b, lmbda, Alu.mult)
                up = tt(b_l, negmu, Alu.subtract)

                out_lb = tt(lb_pre, low, Alu.max)
                out_ub = tt(ub_pre, up, Alu.min)
            else:
                out_lb = lb_pre
                out_ub = ub_pre

            nc.sync.dma_start(olb[:], out_lb[:])
            nc.sync.dma_start(oub[:], out_ub[:])

    nc.compile()
    return nc


_cache: dict = {}


def _get_nc(rows_per_core: int, m: int, concretize: bool, rep: int = 1, **kw):
    key = (rows_per_core, m, concretize, rep, tuple(sorted(kw.items())))
    if key not in _cache:
        _cache[key] = _build(rows_per_core, m, concretize, rep, **kw)
    return _cache[key]


def _make_in_maps(bounds, W, bias, last_bounds, concretize, n_cores):
    rows = W.shape[0] // n_cores if W is not None else bounds.shape[1] // n_cores
    T = rows // 128
    lb, ub = np.asarray(bounds[0], np.float32), np.asarray(bounds[1], np.float32)
    in_maps = []
    if concretize:
        m = W.shape[1]
        JC = m // 128
        plb = np.asarray(last_bounds[0], np.float64)
        pub = np.asarray(last_bounds[1], np.float64)
        c = ((plb + pub) * 0.5).astype(np.float32)
        r = ((pub - plb) * 0.5).astype(np.float32)
        # [128, JC] with element (p, jc) = v[jc*128 + p]
        ct = np.ascontiguousarray(c.astype(np.float16).reshape(JC, 128).T)
        rt = np.ascontiguousarray(r.astype(np.float16).reshape(JC, 128).T)
    for cix in range(n_cores):
        sl = slice(cix * rows, (cix + 1) * rows)
        im = {
            "lbt": np.ascontiguousarray(lb[sl].reshape(T, 128).T),
            "ubt": np.ascontiguousarray(ub[sl].reshape(T, 128).T),
        }
        if concretize:
            im["wt"] = W[sl].T.astype(np.float16)  # [m, rows] contiguous
            im["biast"] = np.ascontiguousarray(
                np.asarray(bias, np.float32)[sl].reshape(T, 128).T
            )
            im["ct"] = ct
            im["rt"] = rt
        in_maps.append(im)
    return in_maps


def _assemble(results, n_cores):
    outs = []
    for cix in range(n_cores):
        o_lb = results[cix]["olb"].T.reshape(-1)  # [T,128] -> rows t*128+p
        o_ub = results[cix]["oub"].T.reshape(-1)
        outs.append(np.stack([o_lb, o_ub]))
    return np.concatenate(outs, axis=1).astype(np.float32)


BEST = dict(wbufs=4)


def kernel(bounds, W, bias, last_bounds, back_sub_steps):
    from concourse.bass_utils import run_bass_kernel_spmd

    bounds = np.asarray(bounds)
    W = np.asarray(W)
    bias = np.asarray(bias)
    last_bounds = np.asarray(last_bounds)
    concretize = int(np.asarray(back_sub_steps)) > 0

    rows = W.shape[0] // N_CORES
    nc = _get_nc(rows, W.shape[1], concretize, **BEST)
    in_maps = _make_in_maps(
        bounds, W if concretize else None, bias, last_bounds, concretize, N_CORES
    )
    res = run_bass_kernel_spmd(nc, in_maps, list(range(N_CORES)))
    return _assemble(res.results, N_CORES)


if __name__ == "__main__":
    rng = np.random.default_rng(0)
    n, m = 1024, 2048  # small smoke (1 core slice = 128 rows)
    bounds = np.sort(rng.standard_normal((2, n)).astype(np.float32), axis=0)
    W = (rng.standard_normal((n, m)) / np.sqrt(m)).astype(np.float32)
    bias = rng.standard_normal(n).astype(np.float32)
    last_bounds = np.sort(rng.standard_normal((2, m)).astype(np.float32), axis=0)
    out = kernel(bounds, W, bias, last_bounds, 1)
    print(out.shape, out.dtype)


# revision 10
# speedup vs baseline: 2.2270x; 1.0570x over previous
"""DeepPoly ReLU transformer (back-substitution concretization) on 8 trn2 cores.

Math (exact rewrite of the reference):
    lb, ub = bounds;  plb, pub = last_bounds
    c = (plb+pub)/2, r = (pub-plb)/2
    s = W @ c,  q = |W| @ r       (identity holds for any sign of r)
    A = s - q  ( = max(W,0)@plb + min(W,0)@pub )
    B = s + q  ( = max(W,0)@pub + min(W,0)@plb )
    ind2 = lb>=0; ind3 = (ub>0)&(lb<0); ind4 = (ub>-lb)&ind3
    beta = 1 if ind2|ind4 else 0
    lmbda = 1 if ind2 else (ub/(ub-lb) if ind3 else 0)
    mu    = -lb*ub/(ub-lb) if ind3 else 0
    low = beta*(A + bias);  up = lmbda*(B + bias) + mu
    out_lb = max(beta*lb, low)
    out_ub = min(where(ind2|ind3, ub, 0), up)

Sharding: rows of W (output neurons) split across 8 cores, 1024 rows each.
Per core the host ships W transposed as fp16 [8192, 1024] (halves HBM
traffic vs fp32 and puts the contraction dim on partitions), so both
matvecs run on the TensorEngine: per [128, 1024] chunk, DVE computes
|Wt| (tensor_scalar abs_max, 4x mode), then PE accumulates
s = Wt.T @ c_chunk and q = |Wt|.T @ r_chunk into PSUM ([1,512] banks,
start/stop over the 64 chunks).  The [1, 1024] PSUM rows are scattered
to [128, 8] by a small SBUF->SBUF DMA and the exact fp32 mask epilogue
(same as the previous DVE version) finishes on [128, 8] tiles.
"""

import sys

sys.path.insert(0, "/opt/trn_rl_repo")

import numpy as np

N_CORES = 8
N = 8192
M = 8192


def _build(
    rows_per_core: int,
    m: int,
    concretize: bool,
    rep: int = 1,
    wbufs: int = 4,
    group: int = 8,
    ablate: str = "",
):
    import contextlib

    import concourse.tile as tile
    from concourse import bacc, mybir

    T = rows_per_core // 128
    assert rows_per_core % 128 == 0
    JC = m // 128  # number of 128-row contraction chunks

    nc = bacc.Bacc("TRN2", target_bir_lowering=False, debug=False)
    f32 = mybir.dt.float32
    f16 = mybir.dt.float16
    Alu = mybir.AluOpType

    lbt = nc.dram_tensor("lbt", [128, T], f32, kind="ExternalInput").ap()
    ubt = nc.dram_tensor("ubt", [128, T], f32, kind="ExternalInput").ap()
    olb = nc.dram_tensor("olb", [128, T], f32, kind="ExternalOutput").ap()
    oub = nc.dram_tensor("oub", [128, T], f32, kind="ExternalOutput").ap()
    if concretize:
        # partition-major: wt[p, jc*rows + i] = W.T[jc*128 + p, i] so each
        # DMA group is 16KB-contiguous per partition (DMA efficiency)
        wt = nc.dram_tensor(
            "wt", [128, JC * rows_per_core], f16, kind="ExternalInput"
        ).ap()
        biast = nc.dram_tensor("biast", [128, T], f32, kind="ExternalInput").ap()
        ct = nc.dram_tensor("ct", [128, JC], f16, kind="ExternalInput").ap()
        rt = nc.dram_tensor("rt", [128, JC], f16, kind="ExternalInput").ap()
        # DRAM scratch for the [1, rows] -> [128, T] partition scatter
        sdram = nc.dram_tensor("sdram", [T, 128], f32, kind="Internal").ap()
        qdram = nc.dram_tensor("qdram", [T, 128], f32, kind="Internal").ap()

    # psum banks per accumulator: free dim <= 512 fp32 (one 2KB bank) each
    bank_slices = [
        slice(b0, min(b0 + 512, rows_per_core))
        for b0 in range(0, rows_per_core, 512)
    ]
    NB = len(bank_slices)

    with tile.TileContext(nc) as tc:
        with (
            tc.tile_pool(name="wp", bufs=wbufs) as wp,
            tc.tile_pool(name="aq", bufs=wbufs) as aq,
            tc.tile_pool(name="ps", bufs=1, space="PSUM") as ps,
            tc.tile_pool(name="bc", bufs=1) as bc,
            tc.tile_pool(name="sm", bufs=1) as sm,
            tc.tile_pool(name="ep", bufs=24) as ep,
            tc.For_i(0, rep, 1) if rep > 1 else contextlib.nullcontext(),
        ):
            lb_s = sm.tile([128, T], f32, tag="lb")
            ub_s = sm.tile([128, T], f32, tag="ub")
            nc.sync.dma_start(lb_s[:], lbt[:])
            nc.sync.dma_start(ub_s[:], ubt[:])

            if concretize:
                ct_s = bc.tile([128, JC], f16, tag="ct")
                rt_s = bc.tile([128, JC], f16, tag="rt")
                nc.sync.dma_start(ct_s[:], ct[:])
                nc.sync.dma_start(rt_s[:], rt[:])
                bias_s = sm.tile([128, T], f32, tag="bias")
                nc.sync.dma_start(bias_s[:], biast[:])

                s_ps = [
                    ps.tile([1, cs.stop - cs.start], f32, name=f"sps{b}", tag=f"sps{b}")
                    for b, cs in enumerate(bank_slices)
                ]
                q_ps = [
                    ps.tile([1, cs.stop - cs.start], f32, name=f"qps{b}", tag=f"qps{b}")
                    for b, cs in enumerate(bank_slices)
                ]

                GF = group * rows_per_core  # free elems per DMA group
                for g in range(JC // group):
                    w = wp.tile([128, GF], f16, tag="w")
                    nc.sync.dma_start(w[:], wt[:, g * GF : (g + 1) * GF])
                    if ablate == "dma":
                        continue
                    if ablate != "smm":
                        a = aq.tile([128, GF], f16, tag="a")
                        # fp16 abs = clear sign bit (int16 view): DVE 4x mode
                        h = GF // 2
                        nc.vector.tensor_scalar(
                            a[:, 0:h].bitcast(mybir.dt.int16),
                            w[:, 0:h].bitcast(mybir.dt.int16),
                            0x7FFF, None, Alu.bitwise_and,
                        )
                        nc.vector.tensor_scalar(
                            a[:, h:GF].bitcast(mybir.dt.int16),
                            w[:, h:GF].bitcast(mybir.dt.int16),
                            0x7FFF, None, Alu.bitwise_and,
                        )
                        if ablate == "dmaabs":
                            continue
                    qsrc = w if ablate == "smm" else a
                    for jcs in range(group):
                        jc = g * group + jcs
                        st, sp = jc == 0, jc == JC - 1
                        o = jcs * rows_per_core
                        for b, cs in enumerate(bank_slices):
                            nc.tensor.matmul(
                                s_ps[b][:], ct_s[:, jc : jc + 1],
                                w[:, o + cs.start : o + cs.stop],
                                start=st, stop=sp,
                            )
                            nc.tensor.matmul(
                                q_ps[b][:], rt_s[:, jc : jc + 1],
                                qsrc[:, o + cs.start : o + cs.stop],
                                start=st, stop=sp,
                            )

                if ablate in ("dma", "dmaabs"):
                    # accumulators never written; fake s/q for the epilogue
                    s_pt = sm.tile([128, T], f32, tag="spt")
                    q_pt = sm.tile([128, T], f32, tag="qpt")
                    nc.vector.memset(s_pt[:], 0.0)
                    nc.vector.memset(q_pt[:], 0.0)
                # PSUM [1, 512] rows -> SBUF [1, rows] -> scatter to [128, T]
                s_row = sm.tile([1, rows_per_core], f32, tag="srow")
                q_row = sm.tile([1, rows_per_core], f32, tag="qrow")
                if ablate in ("dma", "dmaabs"):
                    s_row = None  # sentinel: skip psum drain
                if s_row is not None:
                    for b, cs in enumerate(bank_slices):
                        nc.vector.tensor_copy(s_row[0:1, cs], s_ps[b][:])
                        nc.vector.tensor_copy(q_row[0:1, cs], q_ps[b][:])
                    s_pt = sm.tile([128, T], f32, tag="spt")
                    q_pt = sm.tile([128, T], f32, tag="qpt")
                    nc.sync.dma_start(sdram[:], s_row[0:1, :])
                    nc.sync.dma_start(qdram[:], q_row[0:1, :])
                    nc.sync.dma_start(s_pt[:], sdram.rearrange("t p -> p t"))
                    nc.sync.dma_start(q_pt[:], qdram.rearrange("t p -> p t"))

            # ---------------- epilogue (all fp32, [128, T]) ----------------
            def tt(a, b, op):
                o = ep.tile([128, T], f32)
                nc.vector.tensor_tensor(o[:], a[:], b[:], op=op)
                return o

            def ts(a, s1, op0, s2=None, op1=None):
                o = ep.tile([128, T], f32)
                if op1 is None:
                    nc.vector.tensor_scalar(o[:], a[:], s1, None, op0)
                else:
                    nc.vector.tensor_scalar(o[:], a[:], s1, s2, op0, op1)
                return o

            ind2 = ts(lb_s, 0.0, Alu.is_ge)
            ubpos = ts(ub_s, 0.0, Alu.is_gt)
            lbneg = ts(lb_s, 0.0, Alu.is_lt)
            ind3 = tt(ubpos, lbneg, Alu.mult)
            sumlu = tt(ub_s, lb_s, Alu.add)
            ind4p = ts(sumlu, 0.0, Alu.is_gt)
            ind4 = tt(ind4p, ind3, Alu.mult)
            beta = tt(ind2, ind4, Alu.max)

            lb_pre = tt(beta, lb_s, Alu.mult)
            i23 = tt(ind2, ind3, Alu.max)
            ub_pre = tt(ub_s, i23, Alu.mult)

            if concretize:
                diff = tt(ub_s, lb_s, Alu.subtract)
                dmask = tt(diff, ind3, Alu.mult)
                onemind3 = ts(ind3, -1.0, Alu.mult, 1.0, Alu.add)  # 1 - ind3
                diff_safe = tt(dmask, onemind3, Alu.add)
                rec = ep.tile([128, T], f32)
                nc.vector.reciprocal(rec[:], diff_safe[:])
                ubrec = tt(ub_s, rec, Alu.mult)
                lmb3 = tt(ubrec, ind3, Alu.mult)
                lmbda = tt(ind2, lmb3, Alu.add)
                negmu = tt(lmb3, lb_s, Alu.mult)  # = -mu

                a_lo = tt(s_pt, q_pt, Alu.subtract)
                b_up = tt(s_pt, q_pt, Alu.add)
                a_b = tt(a_lo, bias_s, Alu.add)
                low = tt(a_b, beta, Alu.mult)
                b_b = tt(b_up, bias_s, Alu.add)
                b_l = tt(b_b, lmbda, Alu.mult)
                up = tt(b_l, negmu, Alu.subtract)

                out_lb = tt(lb_pre, low, Alu.max)
                out_ub = tt(ub_pre, up, Alu.min)
            else:
                out_lb = lb_pre
                out_ub = ub_pre

            nc.sync.dma_start(olb[:], out_lb[:])
            nc.sync.dma_start(oub[:], out_ub[:])

    nc.compile()
    return nc


_cache: dict = {}


def _get_nc(rows_per_core: int, m: int, concretize: bool, rep: int = 1, **kw):
    key = (rows_per_core, m, concretize, rep, tuple(sorted(kw.items())))
    if key not in _cache:
        _cache[key] = _build(rows_per_core, m, concretize, rep, **kw)
    return _cache[key]


def _make_in_maps(bounds, W, bias, last_bounds, concretize, n_cores):
    rows = W.shape[0] // n_cores if W is not None else bounds.shape[1] // n_cores
    T = rows // 128
    lb, ub = np.asarray(bounds[0], np.float32), np.asarray(bounds[1], np.float32)
    in_maps = []
    if concretize:
        m = W.shape[1]
        JC = m // 128
        plb = np.asarray(last_bounds[0], np.float64)
        pub = np.asarray(last_bounds[1], np.float64)
        c = ((plb + pub) * 0.5).astype(np.float32)
        r = ((pub - plb) * 0.5).astype(np.float32)
        # [128, JC] with element (p, jc) = v[jc*128 + p]
        ct = np.ascontiguousarray(c.astype(np.float16).reshape(JC, 128).T)
        rt = np.ascontiguousarray(r.astype(np.float16).reshape(JC, 128).T)
    for cix in range(n_cores):
        sl = slice(cix * rows, (cix + 1) * rows)
        im = {
            "lbt": np.ascontiguousarray(lb[sl].reshape(T, 128).T),
            "ubt": np.ascontiguousarray(ub[sl].reshape(T, 128).T),
        }
        if concretize:
            # [128, JC*rows]: wt[p, jc*rows + i] = W[sl][i, jc*128 + p]
            wt = W[sl].T.astype(np.float16)  # [m, rows]
            im["wt"] = np.ascontiguousarray(
                wt.reshape(JC, 128, rows).transpose(1, 0, 2).reshape(128, JC * rows)
            )
            im["biast"] = np.ascontiguousarray(
                np.asarray(bias, np.float32)[sl].reshape(T, 128).T
            )
            im["ct"] = ct
            im["rt"] = rt
        in_maps.append(im)
    return in_maps


def _assemble(results, n_cores):
    outs = []
    for cix in range(n_cores):
        o_lb = results[cix]["olb"].T.reshape(-1)  # [T,128] -> rows t*128+p
        o_ub = results[cix]["oub"].T.reshape(-1)
        outs.append(np.stack([o_lb, o_ub]))
    return np.concatenate(outs, axis=1).astype(np.float32)


BEST = dict(wbufs=3, group=8)


def kernel(bounds, W, bias, last_bounds, back_sub_steps):
    from concourse.bass_utils import run_bass_kernel_spmd

    bounds = np.asarray(bounds)
    W = np.asarray(W)
    bias = np.asarray(bias)
    last_bounds = np.asarray(last_bounds)
    concretize = int(np.asarray(back_sub_steps)) > 0

    rows = W.shape[0] // N_CORES
    nc = _get_nc(rows, W.shape[1], concretize, **BEST)
    in_maps = _make_in_maps(
        bounds, W if concretize else None, bias, last_bounds, concretize, N_CORES
    )
    res = run_bass_kernel_spmd(nc, in_maps, list(range(N_CORES)))
    return _assemble(res.results, N_CORES)


if __name__ == "__main__":
    rng = np.random.default_rng(0)
    n, m = 1024, 2048  # small smoke (1 core slice = 128 rows)
    bounds = np.sort(rng.standard_normal((2, n)).astype(np.float32), axis=0)
    W = (rng.standard_normal((n, m)) / np.sqrt(m)).astype(np.float32)
    bias = rng.standard_normal(n).astype(np.float32)
    last_bounds = np.sort(rng.standard_normal((2, m)).astype(np.float32), axis=0)
    out = kernel(bounds, W, bias, last_bounds, 1)
    print(out.shape, out.dtype)
